# revision 85
# baseline (speedup 1.0000x reference)
"""Trainium2 Bass kernel for nn_Attention_36799279792519.

Full causal self-attention layer (QKV proj + RoPE + causal softmax attention +
output proj), B=2 T=2048 C=1024 H=16 D=64, sharded over 8 NeuronCores:
data-parallel on batch (2) x tensor-parallel on heads (4 heads/core).
Each core computes its heads' attention output and a partial projection
(T, C) in f32; the host sums the 4 partials per batch and adds proj bias.

v2 design (vs the k-outer baseline):
  * q-outer attention: for each 512-wide q chunk j, iterate key blocks kb,
    with per-head score psum tiles (3-buf rotation shared with QKV groups).
  * PV transposed: out[t,d] = at[k,t]^T @ v[k,d] -> [128t, 65] matmuls
    (65 free cols instead of 512 -> half the PE time), the ones-column of V
    accumulates the softmax denominator per PARTITION, so normalization is
    a per-partition reciprocal + tensor_scalar_mul (no DRAM broadcast).
  * normalized [t,d] tiles are PE-transposed back to [d,t] via an identity
    matmul into a bf16 psum tile, then copied to SBUF for the projection.
  * projection accumulates [128,1024] f32 psum per 128-row chunk and DMAs
    it straight to DRAM (f32 output, host converts) - no evac copies.
  * causal masks on the (otherwise idle) GPSIMD/Pool engine.
  * everything pipelined: per-512-col QKV chunks + per-chunk rope/permute;
    attention rounds j=0..3; k/q/v/proj groups for later rounds are
    sprinkled into the attention instruction stream as PE fillers.

Self-contained: hardcodes all shapes; no sibling imports.
"""
import numpy as np
import ml_dtypes

import concourse.bass as bass
import concourse.mybir as mybir
import concourse.tile as tile
from concourse import bacc
from concourse.bass_utils import run_bass_kernel_spmd

B, T, C = 2, 2048, 1024
H, D = 16, 64
SCALE = D ** -0.5
NCORES = 8
CORES_PER_B = NCORES // B          # 4
HPC = H // CORES_PER_B             # 4 heads per core
RL = HPC * D                       # 256 local q/k/v rows
CCH = C // 128                     # 8 contraction chunks
NJ = T // 512                      # 4 q chunks of 512
KB = T // 128                      # 16 key blocks of 128

F32 = mybir.dt.float32
BF16 = mybir.dt.bfloat16
F8 = mybir.dt.float8e4
BF = ml_dtypes.bfloat16

_compiled = {}
DEBUG = False


def _build():
    nc = bacc.Bacc("TRN2", target_bir_lowering=False, debug=False,
                   num_devices=NCORES)

    d = {}
    for nm, shape in (("xh", [C, T]), ("xl", [C, T]),
                      ("wqhl", [C, 2 * RL]), ("wkhl", [C, 2 * RL]),
                      ("wvhl", [C, 2 * RL])):
        d[nm] = nc.dram_tensor(nm, shape, F8, kind="ExternalInput").ap()
    d["wp"] = nc.dram_tensor("wproj_t", [RL, C], BF16, kind="ExternalInput").ap()
    d["bq"] = nc.dram_tensor("bq", [RL], F32, kind="ExternalInput").ap()
    d["bk"] = nc.dram_tensor("bk", [RL], F32, kind="ExternalInput").ap()
    d["bv"] = nc.dram_tensor("bv", [RL], F32, kind="ExternalInput").ap()
    d["ck"] = nc.dram_tensor("cos_k", [128, T], BF16, kind="ExternalInput").ap()
    d["sk"] = nc.dram_tensor("sin_k", [128, T], BF16, kind="ExternalInput").ap()
    d["mask"] = nc.dram_tensor("maskneg", [128, 128], BF16,
                               kind="ExternalInput").ap()
    d["ident"] = nc.dram_tensor("ident", [128, 128], BF16,
                                kind="ExternalInput").ap()
    d["out"] = nc.dram_tensor("out", [T, C], BF16, kind="ExternalOutput").ap()
    if DEBUG:
        for nm in ("dbg_qr0", "dbg_qr1", "dbg_kr0", "dbg_kr1",
                   "dbg_oT0", "dbg_oT1"):
            d[nm] = nc.dram_tensor(nm, [128, T], BF16,
                                   kind="ExternalOutput").ap()
        d["dbg_v"] = nc.dram_tensor("dbg_v", [128, KB * HPC * 65], BF16,
                                    kind="ExternalOutput").ap()
        d["dbg_at"] = nc.dram_tensor("dbg_at", [128, 8, 2, 512], BF16,
                                     kind="ExternalOutput").ap()
        d["dbg_on"] = nc.dram_tensor("dbg_on", [128, 2 * 4 * 64], BF16,
                                     kind="ExternalOutput").ap()
        d["dbg_c"] = nc.dram_tensor("dbg_c", [128, 8, 65], F32,
                                    kind="ExternalOutput").ap()

    with tile.TileContext(nc) as tc:
        _program(nc, tc, d)

    nc.compile()
    return nc


def _program(nc, tc, d):
    AF = mybir.ActivationFunctionType
    with (
        tc.tile_pool(name="const", bufs=1) as const,
        tc.tile_pool(name="qk", bufs=1) as qkpool,
        tc.tile_pool(name="work", bufs=2) as work,
        tc.tile_pool(name="ps_sc", bufs=2, space="PSUM") as ps_sc,
        tc.tile_pool(name="ps_fl", bufs=1, space="PSUM") as ps_fl,
        tc.tile_pool(name="ps_pv", bufs=1, space="PSUM") as ps_pv,
        tc.tile_pool(name="ps_tr", bufs=1, space="PSUM") as ps_tr,
    ):
        # ================= long-lived SBUF tiles =================
        # hi+lo fp8 split of x and the qkv weights (weights pre-scaled x32
        # on the host): x @ w ~= (xh+xl) @ (wh+wl) dropping the lo*lo term.
        # Layout [128, ch, tl, .]: contraction row = ch*256 + tl*128 + p,
        # ready for DoubleRow matmuls (256-row reduction tiles).
        xh_sb = const.tile([128, 4, 2, T], F8)
        xl_sb = const.tile([128, 4, 2, T], F8)
        wq_sb = const.tile([128, 4, 2, 2 * RL], F8, name="wq")
        wk_sb = const.tile([128, 4, 2, 2 * RL], F8, name="wk")
        wv_sb = const.tile([128, 4, 2, 2 * RL], F8, name="wv")
        wp_sb = const.tile([128, 2, C], BF16)
        ck_sb = const.tile([128, T], BF16)
        sk_sb = const.tile([128, T], BF16)
        mask_sb = const.tile([128, 128], BF16)
        ident_sb = const.tile([128, 128], BF16)
        bq_sb = const.tile([128, 2], F32)
        bk_sb = const.tile([128, 2], F32)
        bv_bc = const.tile([128, RL], F32)

        q_ab = [qkpool.tile([128, T], BF16, tag=f"qab{i}", name=f"qab{i}")
                for i in range(2)]
        k_ab = [qkpool.tile([128, T], BF16, tag=f"kab{i}", name=f"kab{i}")
                for i in range(2)]
        # rotated q/k, ab layout: [h*32+i, s, t] = rotated dim s*32+i of
        # head h (s=0: a*cos-b*sin, s=1: a*sin+b*cos)
        qrd = qkpool.tile([128, 2, T], BF16, tag="qrd", name="qrd")
        krd = qkpool.tile([128, 2, T], BF16, tag="krd", name="krd")
        # per-head-contiguous copies for the 64-contraction score matmuls
        qr = [qkpool.tile([128, T], BF16, tag=f"qr{i}", name=f"qr{i}")
              for i in range(2)]
        kr = [qkpool.tile([128, T], BF16, tag=f"kr{i}", name=f"kr{i}")
              for i in range(2)]
        v_sb = qkpool.tile([128, KB, HPC, 65], BF16, tag="v")
        oT = [qkpool.tile([128, T], BF16, tag=f"oT{i}", name=f"oT{i}")
              for i in range(2)]

        nc.vector.memset(v_sb[:, :, :, 64:65], 1.0)

        # ================= input DMAs (ordered for the pipeline) ========
        xh_r = d["xh"].rearrange("(ch tl p) t -> p ch tl t", p=128, tl=2)
        xl_r = d["xl"].rearrange("(ch tl p) t -> p ch tl t", p=128, tl=2)

        def w_r(nm):
            return d[nm].rearrange("(ch tl p) r -> p ch tl r", p=128, tl=2)
        nc.sync.dma_start(out=wk_sb, in_=w_r("wkhl"))
        nc.sync.dma_start(out=xh_sb[:, :, :, 0:512], in_=xh_r[:, :, :, 0:512])
        nc.sync.dma_start(out=wq_sb, in_=w_r("wqhl"))
        nc.sync.dma_start(out=xl_sb[:, :, :, 0:512], in_=xl_r[:, :, :, 0:512])
        nc.sync.dma_start(out=ck_sb[:, 0:512], in_=d["ck"][:, 0:512])
        nc.sync.dma_start(out=sk_sb[:, 0:512], in_=d["sk"][:, 0:512])
        nc.sync.dma_start(out=wv_sb, in_=w_r("wvhl"))
        nc.sync.dma_start(out=mask_sb, in_=d["mask"])
        nc.sync.dma_start(out=ident_sb, in_=d["ident"])
        nc.sync.dma_start(out=bq_sb,
                          in_=d["bq"].rearrange("(rc p) -> p rc", p=128))
        nc.sync.dma_start(out=bk_sb,
                          in_=d["bk"].rearrange("(rc p) -> p rc", p=128))
        nc.sync.dma_start(
            out=bv_bc,
            in_=bass.AP(tensor=d["bv"].tensor, offset=d["bv"].offset,
                        ap=[[0, 128]] + list(d["bv"].ap)))
        for lo, hi in ((512, 1024), (1024, 1536), (1536, 2048)):
            nc.sync.dma_start(out=xh_sb[:, :, :, lo:hi],
                              in_=xh_r[:, :, :, lo:hi])
            nc.sync.dma_start(out=xl_sb[:, :, :, lo:hi],
                              in_=xl_r[:, :, :, lo:hi])
            if lo == 512:
                nc.sync.dma_start(out=ck_sb[:, 512:2048],
                                  in_=d["ck"][:, 512:2048])
                nc.sync.dma_start(out=sk_sb[:, 512:2048],
                                  in_=d["sk"][:, 512:2048])
        nc.sync.dma_start(out=wp_sb,
                          in_=d["wp"].rearrange("(dc p) c -> p dc c", p=128))

        # pull the Exp table load out of the critical path
        warm = work.tile([128, 1], F32, tag="warm", name="warm", bufs=1)
        nc.vector.memset(warm, 0.0)
        nc.scalar.activation(out=warm, in_=warm, func=AF.Exp)

        # ================= emission helpers =================
        DR = mybir.MatmulPerfMode.DoubleRow
        INV32 = 1.0 / 32.0

        def qkv_group(w_sb, b_sb, dst_ab, rc, j, on_act, pro=False):
            lo, hi = j * 512, (j + 1) * 512
            if pro:  # prologue: score psum pool is still free
                ps = ps_sc.tile([128, 2, 512], F32, tag="sc",
                                name=f"qkv{rc}{j}")[:, 0, :]
            else:
                ps = ps_fl.tile([128, 512], F32, tag="fl",
                                name=f"qkv{rc}{j}")
            k = 0
            for wo, b in ((0, xh_sb), (RL, xh_sb), (0, xl_sb)):
                for ch in range(4):
                    nc.tensor.matmul(
                        ps, w_sb[:, ch, :, wo + rc * 128:wo + (rc + 1) * 128],
                        b[:, ch, :, lo:hi],
                        start=(k == 0), stop=(k == 11), perf_mode=DR)
                    k += 1
            if on_act:
                nc.scalar.activation(out=dst_ab[rc][:, lo:hi], in_=ps,
                                     func=AF.Identity, scale=INV32,
                                     bias=b_sb[:, rc:rc + 1])
            else:
                nc.vector.tensor_scalar(
                    dst_ab[rc][:, lo:hi], ps, INV32, b_sb[:, rc:rc + 1],
                    op0=mybir.AluOpType.mult, op1=mybir.AluOpType.add)

        def v_group(kb):
            ps = ps_fl.tile([128, 512], F32, tag="fl", name=f"v{kb}")
            psv = ps[:, 0:RL]
            k = 0
            for a, wo in ((xh_sb, 0), (xh_sb, RL), (xl_sb, 0)):
                for ch in range(4):
                    nc.tensor.matmul(
                        psv, a[:, ch, :, kb * 128:(kb + 1) * 128],
                        wv_sb[:, ch, :, wo:wo + RL],
                        start=(k == 0), stop=(k == 11), perf_mode=DR)
                    k += 1
            nc.vector.scalar_tensor_tensor(
                out=v_sb[:, kb, :, 0:64],
                in0=psv.rearrange("p (h dd) -> p h dd", h=HPC),
                scalar=INV32,
                in1=bv_bc.rearrange("p (h dd) -> p h dd", h=HPC),
                op0=mybir.AluOpType.mult,
                op1=mybir.AluOpType.add)

        def rope_chunk(ab, rd, j):
            lo, hi = j * 512, (j + 1) * 512
            t1 = work.tile([128, 512], BF16, tag="rt1", name="rt1")
            t2 = work.tile([128, 512], BF16, tag="rt2", name="rt2")
            nc.vector.tensor_mul(t1, ab[0][:, lo:hi], ck_sb[:, lo:hi])
            nc.vector.tensor_mul(t2, ab[1][:, lo:hi], sk_sb[:, lo:hi])
            nc.vector.tensor_sub(rd[:, 0, lo:hi], t1, t2)
            t3 = work.tile([128, 512], BF16, tag="rt1", name="rt3")
            t4 = work.tile([128, 512], BF16, tag="rt2", name="rt4")
            nc.vector.tensor_mul(t3, ab[0][:, lo:hi], sk_sb[:, lo:hi])
            nc.vector.tensor_mul(t4, ab[1][:, lo:hi], ck_sb[:, lo:hi])
            nc.vector.tensor_add(rd[:, 1, lo:hi], t3, t4)

        def permute_chunk(rd, dst, j):
            # one DMA per pair: [64, 2, w] -> [128, w] in flat run order,
            # leaving each head's 64 dims INTERLEAVED (d0t, d0b, d1t, ...).
            # Scores only contract over these rows, and q and k share the
            # same order, so the interleave is harmless.
            lo, hi = j * 512, (j + 1) * 512
            for pair in range(2):
                nc.sync.dma_start(
                    out=dst[pair][:, lo:hi],
                    in_=rd[pair * 64:(pair + 1) * 64, :, lo:hi])

        def k_chunk(j, on_act=False, pro=False):
            qkv_group(wk_sb, bk_sb, k_ab, 0, j, on_act, pro)
            qkv_group(wk_sb, bk_sb, k_ab, 1, j, False, pro)
            rope_chunk(k_ab, krd, j)

        def q_chunk(j, on_act=False, pro=False):
            qkv_group(wq_sb, bq_sb, q_ab, 0, j, on_act, pro)
            qkv_group(wq_sb, bq_sb, q_ab, 1, j, False, pro)
            rope_chunk(q_ab, qrd, j)

        def proj_half(t16, half, tail=False):
            if tail:  # score psum pool is free after the last exp
                ps = ps_sc.tile([128, 2, 512], F32, tag="sc",
                                name=f"pj{t16}{half}")[:, 0, :]
            else:
                ps = ps_fl.tile([128, 512], F32, tag="fl",
                                name=f"pj{t16}{half}")
            for dc in range(2):
                nc.tensor.matmul(
                    ps,
                    oT[dc][:, t16 * 128:(t16 + 1) * 128],
                    wp_sb[:, dc, half * 512:(half + 1) * 512],
                    start=(dc == 0), stop=(dc == 1))
            o_sb = osb_tiles[t16 % 2]
            if tail and half == 0:
                nc.scalar.copy(o_sb[:, half * 512:(half + 1) * 512], ps)
            else:
                nc.vector.tensor_copy(o_sb[:, half * 512:(half + 1) * 512],
                                      ps)
            if half == 1:
                nc.sync.dma_start(
                    out=d["out"][t16 * 128:(t16 + 1) * 128, :], in_=o_sb)

        osb_tiles = [qkpool.tile([128, C], BF16, tag=f"osb{i}",
                                 name=f"osb{i}") for i in range(2)]

        # ============ software-pipelined attention + fills ============
        fills = []
        iters = []
        for j in range(NJ):
            for pair in range(2):
                for kb in range(4 * j + 4):
                    iters.append((pair, j, kb))

        state = {}  # (pair, j) -> dict(pv, at map, rec, o_n)

        def emit_scores(it):
            pair, j, kb = it
            q0 = 512 * j
            k0 = kb * 128
            qlo = max(q0, k0)
            w = 512 - (qlo - q0)
            st = state.setdefault((pair, j), {})
            if "pv" not in st:
                st["pv"] = ps_pv.tile([128, 2, 512], F32, tag="pv",
                                      name=f"pv{pair}{j}")
                st["rec"] = work.tile([128, 2, 4, 1], F32, tag="rec",
                                      name=f"rec{pair}{j}")
                st["o_n"] = work.tile([128, 2, 4, 64], BF16, tag="on",
                                      name=f"on{pair}{j}")
            ps = ps_sc.tile([128, 2, 512], F32, tag="sc",
                            name=f"sc{pair}{j}{kb}")
            diag = kb >= 4 * j
            for hh in range(2):
                h = 2 * pair + hh
                if j == 0:
                    # chunk-0 q/k not yet permuted per-head: contract the
                    # two rope halves separately from the ab-layout tile
                    for s in range(2):
                        nc.tensor.matmul(
                            ps[:, hh, 0:w],
                            krd[h * 32:(h + 1) * 32, s, k0:k0 + 128],
                            qrd[h * 32:(h + 1) * 32, s, qlo:q0 + 512],
                            start=(s == 0), stop=False,
                            tile_position=(h * 32, 0))
                else:
                    nc.tensor.matmul(
                        ps[:, hh, 0:w],
                        kr[pair][hh * 64:(hh + 1) * 64, k0:k0 + 128],
                        qr[pair][hh * 64:(hh + 1) * 64, qlo:q0 + 512],
                        start=True, stop=not diag)
                if diag:  # add -100/SCALE above the diagonal, then exp -> 0
                    nc.tensor.matmul(ps[:, hh, 0:128], ident_sb, mask_sb,
                                     start=False, stop=True)
            at = work.tile([128, 2, 512], BF16, tag=f"at{kb}",
                           name=f"at{pair}{j}{kb}", bufs=2)
            nc.scalar.activation(out=at[:, :, 0:w], in_=ps[:, :, 0:w],
                                 func=AF.Exp, scale=float(SCALE))
            st[kb] = at
            if DEBUG and pair == 0 and j == 1:
                nc.sync.dma_start(
                    out=d["dbg_at"][:, kb, :, 0:w],
                    in_=at[:, :, 0:w])

        def emit_pv(it):
            # PSUM zero regions (banks) allow only ONE live accumulation
            # group: per head-bank, the tq sub-chunks accumulate
            # SEQUENTIALLY into pv[:, hh, 0:65].  tq=0 streams along with
            # the exps; tq=m>=1 replays the round's cached `at` tiles in a
            # burst at diagonal iteration m.
            pair, j, kb = it
            q0 = 512 * j
            st = state[(pair, j)]
            pv = st["pv"]
            m = kb - 4 * j
            if kb <= 4 * j:  # tq0 streaming group
                at = st[kb]
                for hh in range(2):
                    nc.tensor.matmul(
                        pv[:, hh, 0:65], at[:, hh, 0:128],
                        v_sb[:, kb, 2 * pair + hh, :],
                        start=(kb == 0), stop=(kb == 4 * j))
            if m >= 1:  # replay sweep for sub-chunk tq = m
                tqq = q0 + 128 * m
                for hh in range(2):
                    h = 2 * pair + hh
                    for kb2 in range(4 * j + m + 1):
                        qlo2 = max(q0, kb2 * 128)
                        nc.tensor.matmul(
                            pv[:, hh, 0:65],
                            st[kb2][:, hh, tqq - qlo2:tqq - qlo2 + 128],
                            v_sb[:, kb2, h, :],
                            start=(kb2 == 0), stop=(kb2 == 4 * j + m))
                if DEBUG and pair == 0 and j == 1 and m == 3:
                    for kb2 in range(8):
                        qlo2 = max(q0, kb2 * 128)
                        dps = ps_fl.tile([128, 512], F32, tag="fl",
                                         name=f"dbg{kb2}")
                        nc.tensor.matmul(
                            dps[:, 0:65],
                            st[kb2][:, 0, tqq - qlo2:tqq - qlo2 + 128],
                            v_sb[:, kb2, 0, :], start=True, stop=True)
                        dsb = work.tile([128, 65], F32, tag="dbgc",
                                        name=f"dbgc{kb2}")
                        nc.vector.tensor_copy(dsb, dps[:, 0:65])
                        nc.sync.dma_start(out=d["dbg_c"][:, kb2, :], in_=dsb)
            if m >= 0:  # sub-chunk tq = m is complete: normalize it now
                for hh in range(2):
                    nc.vector.reciprocal(st["rec"][:, hh, m],
                                         pv[:, hh, 64:65])
                    nc.vector.tensor_scalar_mul(
                        st["o_n"][:, hh, m, :], pv[:, hh, 0:64],
                        st["rec"][:, hh, m, :])
                if m == 0:
                    st["tr"] = ps_tr.tile([128, 512], BF16, tag="tr",
                                          name=f"tr{pair}{j}")
                pend_tr.append((pair, j, m))
                if DEBUG and pair == 0 and j == 1 and m == 3:
                    nc.sync.dma_start(
                        out=d["dbg_on"],
                        in_=st["o_n"].rearrange("p a b c -> p (a b c)"))
                if m == 3:
                    for kb2 in range(4 * j + 4):
                        st.pop(kb2)

        def emit_transposes(pair, j, tq):
            st = state[(pair, j)]
            for hh in range(2):
                nc.tensor.transpose(
                    st["tr"][hh * 64:(hh + 1) * 64,
                             tq * 128:(tq + 1) * 128],
                    st["o_n"][:, hh, tq, :], ident_sb)
            nc.vector.tensor_copy(
                oT[pair][:, 512 * j + 128 * tq:512 * j + 128 * (tq + 1)],
                st["tr"][:, tq * 128:(tq + 1) * 128])
            if pair == 1 and j == NJ - 1:
                # last round: project this 128-row chunk immediately
                proj_half(4 * j + tq, 0, tail=True)
                proj_half(4 * j + tq, 1, tail=True)

        # ================= schedule =================
        # prologue: chunk-0 k/q (no permute: round 0 contracts the rope
        # halves directly) + first v blocks
        k_chunk(0, on_act=True, pro=True)
        q_chunk(0, on_act=True, pro=True)
        for kb in range(4):
            v_group(kb)

        round_fills = {j: [] for j in range(NJ)}

        def k_fills(jj):
            def k_rest():
                qkv_group(wk_sb, bk_sb, k_ab, 1, jj, False)
                rope_chunk(k_ab, krd, jj)
                permute_chunk(krd, kr, jj)
            return [lambda: qkv_group(wk_sb, bk_sb, k_ab, 0, jj, False),
                    k_rest]

        def q_fills(jj):
            def q_rest():
                qkv_group(wq_sb, bq_sb, q_ab, 1, jj, False)
                rope_chunk(q_ab, qrd, jj)
                permute_chunk(qrd, qr, jj)
            return [lambda: qkv_group(wq_sb, bq_sb, q_ab, 0, jj, False),
                    q_rest]

        def v_fills(kbs):
            return [lambda kk=kb: v_group(kk) for kb in kbs]

        def p_fills(t16s):
            out = []
            for t16 in t16s:
                out.append(lambda tt=t16: proj_half(tt, 0))
                out.append(lambda tt=t16: proj_half(tt, 1))
            return out

        # placement balances per-round PE fill work against the growing
        # exp (Act) load; chunk k(j)/q(j) must precede round j's scores --
        # except k3's high blocks, first read at iteration 12 of round 3
        round_fills[0] = ([lambda: permute_chunk(krd, kr, 0)]
                          + k_fills(1) + q_fills(1) + v_fills(range(4, 8)))
        round_fills[1] = k_fills(2) + q_fills(2)
        round_fills[2] = q_fills(3) + v_fills(range(8, 12)) + p_fills(range(3))
        round_fills[3] = (k_fills(3) + v_fills(range(12, 16))
                          + p_fills(range(3, 12)))

        # interleaved emission: scores(i+1) before pv(i); transposes for
        # sub-chunk tq lag one iteration behind their normalize
        pend_tr = []
        emit_scores(iters[0])
        emit_scores(iters[1])
        for i, it in enumerate(iters):
            if i + 2 < len(iters):
                emit_scores(iters[i + 2])
            for jj in range(it[1] + 1):  # earliest unfinished fills first
                if round_fills[jj]:
                    round_fills[jj].pop(0)()
                    break
            if pend_tr:
                emit_transposes(*pend_tr.pop(0))
            emit_pv(it)
        while pend_tr:
            emit_transposes(*pend_tr.pop(0))
        for j in range(NJ):
            while round_fills[j]:
                round_fills[j].pop(0)()

        if DEBUG:
            nc.sync.dma_start(out=d["dbg_qr0"][:, 512:], in_=qr[0][:, 512:])
            nc.sync.dma_start(out=d["dbg_qr1"][:, 512:], in_=qr[1][:, 512:])
            nc.sync.dma_start(out=d["dbg_kr0"], in_=kr[0])
            nc.sync.dma_start(out=d["dbg_kr1"], in_=kr[1])
            nc.sync.dma_start(out=d["dbg_oT0"], in_=oT[0])
            nc.sync.dma_start(out=d["dbg_oT1"], in_=oT[1])
            nc.sync.dma_start(
                out=d["dbg_v"],
                in_=v_sb.rearrange("p a b c -> p (a b c)"))


F8NP = ml_dtypes.float8_e4m3fn


def _split8(a):
    """bf16-valued f32 array -> (hi, lo) fp8 pair with a ~= hi + lo."""
    hi = a.astype(F8NP)
    lo = (a - hi.astype(np.float32)).astype(F8NP)
    return np.ascontiguousarray(hi), np.ascontiguousarray(lo)


def _host_prep(hidden_states, cos, sin, qkv_w, qkv_b, proj_w):
    cos_rep = np.tile(np.ascontiguousarray(cos.T), (HPC, 1))
    sin_rep = np.tile(np.ascontiguousarray(sin.T), (HPC, 1))
    ck = cos_rep.astype(BF)
    sk = sin_rep.astype(BF)
    maskneg = np.where(np.arange(128)[:, None] > np.arange(128)[None, :],
                       np.float32(-100.0 / SCALE), np.float32(0.0)).astype(BF)
    ident = np.eye(128, dtype=BF)

    xs = [None, None]
    for b in range(B):
        xbf = hidden_states[b].T.astype(BF).astype(np.float32)
        xs[b] = _split8(xbf)

    in_maps = []
    for c in range(NCORES):
        b = c // CORES_PER_B
        h0 = (c % CORES_PER_B) * HPC
        heads = list(range(h0, h0 + HPC))
        ev = [h * D + 2 * j for h in heads for j in range(D // 2)]
        od = [h * D + 2 * j + 1 for h in heads for j in range(D // 2)]
        perm = ev + od
        vrows = [h * D + dd for h in heads for dd in range(D)]

        def wsplit(wmat):  # [C, RL] f32, pre-scaled x32 via bf16 values
            wbf = (wmat.astype(BF).astype(np.float32)) * 32.0
            return _split8(wbf)

        wqh, wql = wsplit(qkv_w[0 * H * D:1 * H * D][perm].T)
        wkh, wkl = wsplit(qkv_w[1 * H * D:2 * H * D][perm].T)
        wvh, wvl = wsplit(qkv_w[2 * H * D:3 * H * D][vrows].T)
        in_maps.append(dict(
            xh=xs[b][0], xl=xs[b][1],
            wqhl=np.ascontiguousarray(np.concatenate([wqh, wql], axis=1)),
            wkhl=np.ascontiguousarray(np.concatenate([wkh, wkl], axis=1)),
            wvhl=np.ascontiguousarray(np.concatenate([wvh, wvl], axis=1)),
            wproj_t=np.ascontiguousarray(proj_w[:, vrows].T).astype(BF),
            bq=np.ascontiguousarray(qkv_b[0 * H * D:1 * H * D][perm]),
            bk=np.ascontiguousarray(qkv_b[1 * H * D:2 * H * D][perm]),
            bv=np.ascontiguousarray(qkv_b[2 * H * D:3 * H * D][vrows]),
            cos_k=ck, sin_k=sk, maskneg=maskneg, ident=ident,
        ))
    return in_maps


def kernel(hidden_states, cos, sin, qkv_w, qkv_b, proj_w, proj_b):
    hidden_states = np.asarray(hidden_states, dtype=np.float32)
    cos = np.asarray(cos, dtype=np.float32)
    sin = np.asarray(sin, dtype=np.float32)
    qkv_w = np.asarray(qkv_w, dtype=np.float32)
    qkv_b = np.asarray(qkv_b, dtype=np.float32)
    proj_w = np.asarray(proj_w, dtype=np.float32)
    proj_b = np.asarray(proj_b, dtype=np.float32)

    if "nc" not in _compiled:
        _compiled["nc"] = _build()
    nc = _compiled["nc"]

    in_maps = _host_prep(hidden_states, cos, sin, qkv_w, qkv_b, proj_w)
    res = run_bass_kernel_spmd(nc, in_maps, core_ids=list(range(NCORES)))
    outs = [np.asarray(res.results[c]["out"], dtype=np.float32)
            for c in range(NCORES)]
    final = np.empty((B, T, C), np.float32)
    for b in range(B):
        acc = outs[b * CORES_PER_B].copy()
        for i in range(1, CORES_PER_B):
            acc += outs[b * CORES_PER_B + i]
        final[b] = acc + proj_b[None, :]
    return final
